# revision 7
# baseline (speedup 1.0000x reference)
"""Trainium2 Bass kernel for nn_DistanceEdgeSelfCond.

Computes, for inputs pred_coords [8,512,3], mask [8,512], W [64,32], b [64]:
    d[i,j]   = ||x_i - x_j||                        (pairwise distances)
    rbf      = exp(coeff * (d - o_k)^2)             (gaussian smearing, K=32)
    edge     = rbf @ W.T + b                        ([B,512,512,64])
    out      = edge * (mask_i * mask_j)[...,None]

Sharding: data-parallel over B — one batch per NeuronCore (8 cores).

Device pipeline (per core):
  1. Gram matmul (fp32) with host-augmented [5, nc] factors -> d^2 chunks
     of 124 i-rows; DVE relu + ACT sqrt -> d fp32; Pool shifts by -6 and
     casts to fp16 (the shift centers the rbf-active region so fp16
     rounding of d stays ~1.6e-3 there). Partitions 124/125 of each
     chunk hold constant ones-rows (filled by DMA; engine ops cannot
     start at partition 124).
  2. Per 4-i-row half: ONE fp16 broadcast matmul. The select matrix
     carries 1.0 indicators on the d-rows plus hi/lo halves of
     -(o_k - 6) against the two ones-rows, so PSUM receives
     diff = d - o_k (exact center) replicated over the 32 rbf
     channels: [(i_sub,k), j].
  3. ACT Derivative_Erf: d/dx erf = 2/sqrt(pi) * exp(-x^2), so ONE
     activation with scale=sqrt(-coeff) turns diff directly into
     (2/sqrt(pi)) * rbf in fp16 — no separate square or exp pass.
     The sqrt(pi)/2 factor is folded into W on the host.
  4. Edge matmul fp16 per pixel-offset e in 0..7: lhsT = rbf block e
     (columns stored e-major so weights APs stay contiguous), rhs =
     block-diagonal W' -> out [(half,p), (i_sub,d)] fp32 PSUM; each
     output partition owns 8 consecutive pixels.
  5. Evac = pure fp32->fp16 convert copies split DVE/ACT (bias b is
     added on the host; Pool/GPSIMD cannot access PSUM on TRN2).
  6. fp16 stage -> HBM as fully-contiguous 4 KiB-per-partition slabs in
     device order [bb, (half,p), (g,e,d)]; the host reorders to
     [i, j, d] and upcasts to fp32.

Walrus's PE LDWEIGHTS struct carries at most ONE sync wait, so a
post-pass relocates excess waits onto InstNoOp instructions inserted
immediately before in the same engine stream.
"""

import sys

import numpy as np

for _p in ("/opt/trn_rl_repo", "/root/.axon_site/_ro/trn_rl_repo"):
    if _p not in sys.path:
        sys.path.append(_p)

B = 8
N = 512
K = 32
D = 64
CUTOFF = 10.0
DSHIFT = 6.0

CHUNK = 124          # d rows per chunk (partitions 124/125 = ones-rows)
NCHUNK = 5           # 124*4 + 16
HALVES_PER_CHUNK = CHUNK // 4  # 31

_CACHE = {}
TRACE = False  # set True (e.g. from test.py) to capture an NTFF profile


def _fix_waits(nc, mybir):
    """Enforce <=1 embedded sync wait on compute-engine instructions.

    Walrus's per-instruction ISA structs (PE S3_LW, DVE/ACT S2S2D2_*)
    carry a single sync-wait slot.  Excess waits move onto InstNoOp
    instructions inserted immediately before the instruction in the same
    engine stream — gating an earlier point of the same engine is
    strictly more conservative, and with no instruction in between it
    cannot deadlock.
    """
    limited = {
        mybir.EngineType.PE,
        mybir.EngineType.DVE,
        mybir.EngineType.Activation,
        mybir.EngineType.SP,
        mybir.EngineType.Pool,
    }
    for blk in nc.m.functions[0].blocks:
        insts = blk.instructions
        i = 0
        while i < len(insts):
            inst = insts[i]
            si = inst.sync_info
            if (
                inst.engine in limited
                and si is not None
                and si.on_wait
                and len(si.on_wait) > 1
            ):
                waits = list(si.on_wait)
                excess, keep = waits[:-1], waits[-1:]
                for w in excess:
                    nop = mybir.InstNoOp(
                        name=nc.get_next_instruction_name(),
                        sync_info=mybir.SyncInfo(on_wait=[w], on_update=[]),
                        bass_nofuse=True,
                        engine=inst.engine,
                    )
                    nc.register_instruction(nop)
                    insts.insert(i, nop)
                    i += 1
                si.on_wait = keep
            i += 1


def _half_chunk(hh):
    """half index (4 i-rows) -> (chunk c, local half index lr)."""
    if hh < 4 * HALVES_PER_CHUNK:
        return hh // HALVES_PER_CHUNK, hh % HALVES_PER_CHUNK
    return 4, hh - 4 * HALVES_PER_CHUNK


# ct16 column offsets: sel [128, 31*128], wc [128, 256], ones [124:126, 512]
C16_SEL = 0
C16_WC = HALVES_PER_CHUNK * 128          # 3968
C16_ONE = C16_WC + 256                   # 4224
CW16 = C16_ONE + 512                     # 4736


def _build_program():
    import concourse.bass as bass
    import concourse.tile as tile
    from concourse import mybir

    f32 = mybir.dt.float32
    f16 = mybir.dt.float16
    AF = mybir.ActivationFunctionType

    o = np.linspace(0.0, CUTOFF, K)
    coeff = float(-0.5 / (o[1] - o[0]) ** 2)
    s_scale = float(np.sqrt(-coeff))

    nc = bass.Bass("TRN2", target_bir_lowering=False, debug=False)

    ct32_d = nc.dram_tensor("ct32", [5, 1024], f32, kind="ExternalInput")
    ct16_d = nc.dram_tensor("ct16", [128, CW16], f16, kind="ExternalInput")
    out_d = nc.dram_tensor("out", [N // 8, 128, 2048], f16, kind="ExternalOutput")

    with tile.TileContext(nc) as tc:
        with (
            tc.tile_pool(name="consts", bufs=1) as consts,
            tc.tile_pool(name="dtile", bufs=1) as dpool,
            tc.tile_pool(name="work", bufs=4) as work,
            tc.tile_pool(name="stage", bufs=4) as stpool,
            tc.tile_pool(name="psA", bufs=2, space=bass.MemorySpace.PSUM) as psA,
            tc.tile_pool(name="psB", bufs=2, space=bass.MemorySpace.PSUM) as psB,
        ):
            ct32_s = consts.tile([128, 1024], f32, tag="ct32")
            ct16_s = consts.tile([128, CW16], f16, tag="ct16")
            a32 = ct32_d.ap()
            a16 = ct16_d.ap()
            nc.sync.dma_start(ct32_s[0:5, :], a32)
            # first sel blocks land before the bulk
            nc.sync.dma_start(ct16_s[:, 0:1024], a16[:, 0:1024])
            nc.sync.dma_start(ct16_s[:, 1024:CW16], a16[:, 1024:CW16])
            lg_s = ct32_s[0:5, 0:N]
            rg_s = ct32_s[0:5, N : 2 * N]
            sel_s = ct16_s[:, C16_SEL : C16_SEL + HALVES_PER_CHUNK * 128]
            wc_s = ct16_s[:, C16_WC : C16_WC + 256]

            # Phase 1: d chunks [124 rows, 512 j] fp16 (shifted by -6)
            dext = [
                dpool.tile([128, N], f16, name=f"dx{c}", tag=f"dx{c}")
                for c in range(NCHUNK)
            ]
            for c in range(NCHUNK):
                nr = CHUNK if c < 4 else N - 4 * CHUNK
                if nr < CHUNK:
                    # zero the unwritten rows so sel's 0.0 entries never
                    # multiply uninitialized NaN bits
                    nc.vector.memset(dext[c][:], 0.0)
                gps = psB.tile([128, 2 * N], f32, tag="eps")
                nc.tensor.matmul(
                    gps[0:nr, 0:N], lg_s[:, c * CHUNK : c * CHUNK + nr], rg_s
                )
                draw = work.tile([128, N], f32, tag="draw")
                nc.vector.tensor_scalar_max(draw[0:nr, :], gps[0:nr, 0:N], 0.0)
                dfull = work.tile([128, N], f32, tag="dfull")
                nc.scalar.activation(dfull[0:nr, :], draw[0:nr, :], AF.Sqrt)
                nc.gpsimd.tensor_scalar_add(
                    dext[c][0:nr, :], dfull[0:nr, :], -DSHIFT
                )
                nc.sync.dma_start(
                    dext[c][CHUNK : CHUNK + 2, :],
                    a16[CHUNK : CHUNK + 2, C16_ONE : C16_ONE + 512],
                )

            # Phase 2
            diff_tiles = {}

            def emit_bcast(bb):
                diff = psA.tile([128, 2 * N], f32, tag="diff")
                for bi2 in range(2):
                    c, lr = _half_chunk(2 * bb + bi2)
                    nc.tensor.matmul(
                        diff[:, bi2 * N : (bi2 + 1) * N],
                        sel_s[0:126, lr * 128 : (lr + 1) * 128],
                        dext[c][0:126, :],
                    )
                diff_tiles[bb] = diff

            NBB = N // 8
            evac_ctr = 0
            emit_bcast(0)
            for bb in range(NBB):
                if bb + 1 < NBB:
                    emit_bcast(bb + 1)
                diff = diff_tiles.pop(bb)
                # rbf stored e-major: [q, (e, b, p)] so each edge matmul's
                # weights AP is a contiguous [128, 128] block
                rbf = work.tile([128, 2 * N], f16, tag="rbf")
                rbf_w = rbf[:].rearrange("q (e b p) -> q b p e", e=8, b=2, p=64)
                dif_v = diff[:].rearrange("q (b p e) -> q b p e", b=2, p=64, e=8)
                nc.scalar.activation(
                    rbf_w, dif_v, AF.Derivative_Erf, scale=s_scale
                )

                stage = stpool.tile([128, 2048], f16, tag="stage")
                stv = stage[:].rearrange("m (g e d) -> m g e d", g=4, e=8)
                for t in range(2):
                    eps = psB.tile([128, 2 * N], f32, tag="eps")
                    for el in range(4):
                        e = 4 * t + el
                        nc.tensor.matmul(
                            eps[:, el * 256 : (el + 1) * 256],
                            rbf[:, e * 128 : (e + 1) * 128],
                            wc_s,
                        )
                    src = eps[:].rearrange("q (el g d) -> q g el d", el=4, g=4)
                    dst = stv[:, :, 4 * t : 4 * t + 4, :]
                    # Pool cannot read PSUM; rotate the convert-copies over
                    # DVE (2/3) and ACT (1/3, alongside its Derivative_Erf)
                    if evac_ctr % 3 == 2:
                        nc.scalar.activation(dst, src, AF.Copy)
                    else:
                        nc.vector.tensor_copy(dst, src)
                    evac_ctr += 1
                nc.sync.dma_start(out_d.ap()[bb], stage[:])

    _fix_waits(nc, mybir)
    return nc


def _host_inputs(pred_coords):
    x64 = pred_coords.astype(np.float64)  # [B, N, 3]
    r = (x64 * x64).sum(-1)  # [B, N]
    ones = np.ones((B, N), np.float64)
    lg = np.stack(
        [x64[:, :, 0], x64[:, :, 1], x64[:, :, 2], r, ones], axis=1
    ).astype(np.float32)  # [B, 5, N]
    rg = np.stack(
        [-2 * x64[:, :, 0], -2 * x64[:, :, 1], -2 * x64[:, :, 2], ones, r],
        axis=1,
    ).astype(np.float32)  # [B, 5, N]
    return lg, rg


def _host_consts(W):
    o = np.linspace(0.0, CUTOFF, K)

    sel = np.zeros((128, HALVES_PER_CHUNK * 128), np.float16)
    m = np.arange(128)
    bias = -(o[m % 32] - DSHIFT)  # f64
    b_hi = bias.astype(np.float16)
    b_lo = (bias - b_hi.astype(np.float64)).astype(np.float16)
    for lr in range(HALVES_PER_CHUNK):
        sel[4 * lr + m // 32, lr * 128 + m] = np.float16(1.0)
        sel[124, lr * 128 + m] = b_hi
        sel[125, lr * 128 + m] = b_lo

    # sqrt(pi)/2 compensates Derivative_Erf's 2/sqrt(pi) prefactor
    wc = np.zeros((128, 256), np.float16)
    wt = (W.astype(np.float64).T * (np.sqrt(np.pi) / 2.0)).astype(np.float16)
    for g in range(4):
        wc[32 * g : 32 * (g + 1), 64 * g : 64 * (g + 1)] = wt

    ct16 = np.zeros((128, CW16), np.float16)
    ct16[:, C16_SEL : C16_SEL + HALVES_PER_CHUNK * 128] = sel
    ct16[:, C16_WC : C16_WC + 256] = wc
    ct16[124:126, C16_ONE : C16_ONE + 512] = np.float16(1.0)
    return ct16


def kernel(pred_coords, mask, W, b):
    from concourse.bass_utils import run_bass_kernel_spmd

    pred_coords = np.asarray(pred_coords)
    mask = np.asarray(mask)
    W = np.asarray(W)
    b = np.asarray(b)

    if "nc" not in _CACHE:
        _CACHE["nc"] = _build_program()
    nc = _CACHE["nc"]

    lg, rg = _host_inputs(pred_coords)
    ct16 = _host_consts(W)
    in_maps = []
    for cidx in range(B):
        ct32 = np.concatenate([lg[cidx], rg[cidx]], axis=1).astype(np.float32)
        in_maps.append({"ct32": ct32, "ct16": ct16})
    import os

    tdir = os.environ.get("KTRACE_DIR") or None
    res = run_bass_kernel_spmd(
        nc, in_maps, list(range(B)), trace=TRACE, tmpdir=tdir
    )
    _CACHE["last_res"] = res
    # device order [bb, (bi2, p), (g, e, d)] -> [i, j, d]
    outs = []
    for c in range(B):
        arr = np.asarray(res.results[c]["out"])  # [64, 128, 2048] f16
        arr = arr.reshape(64, 2, 64, 4, 8, 64)
        arr = arr.transpose(0, 1, 3, 2, 4, 5).reshape(N, N, D)
        outs.append(arr)
    out = np.stack(outs).astype(np.float32) + b.astype(np.float32)

    if not np.all(mask == 1.0):
        adj = (mask[:, None, :] * mask[:, :, None]).astype(np.float32)
        out = out * adj[..., None]
    return out


# revision 8
# speedup vs baseline: 2.4761x; 2.4761x over previous
"""Trainium2 Bass kernel for nn_DistanceEdgeSelfCond.

Computes, for inputs pred_coords [8,512,3], mask [8,512], W [64,32], b [64]:
    d[i,j]   = ||x_i - x_j||                        (pairwise distances)
    rbf      = exp(coeff * (d - o_k)^2)             (gaussian smearing, K=32)
    edge     = rbf @ W.T + b                        ([B,512,512,64])
    out      = edge * (mask_i * mask_j)[...,None]

Sharding: data-parallel over B — one batch per NeuronCore (8 cores).

Device pipeline (per core):
  1. Gram matmul (fp32) with host-augmented [5, nc] factors -> d^2 chunks
     of 124 i-rows; DVE relu + ACT sqrt -> d fp32; Pool shifts by -6 and
     casts to fp16 (the shift centers the rbf-active region so fp16
     rounding of d stays ~1.6e-3 there). Partitions 124/125 of each
     chunk hold constant ones-rows (filled by DMA; engine ops cannot
     start at partition 124).
  2. Per 4-i-row half: ONE fp16 broadcast matmul. The select matrix
     carries 1.0 indicators on the d-rows plus hi/lo halves of
     -(o_k - 6) against the two ones-rows, so PSUM receives
     diff = d - o_k (exact center) replicated over the 32 rbf
     channels: [(i_sub,k), j].
  3. ACT Derivative_Erf: d/dx erf = 2/sqrt(pi) * exp(-x^2), so ONE
     activation with scale=sqrt(-coeff) turns diff directly into
     (2/sqrt(pi)) * rbf in fp16 — no separate square or exp pass.
     The sqrt(pi)/2 factor is folded into W on the host.
  4. Edge matmul fp16 per pixel-offset e in 0..7: lhsT = rbf block e
     (columns stored e-major so weights APs stay contiguous), rhs =
     block-diagonal W' -> out [(half,p), (i_sub,d)] fp32 PSUM; each
     output partition owns 8 consecutive pixels.
  5. Evac = pure fp32->fp16 convert copies split DVE/ACT (bias b is
     added on the host; Pool/GPSIMD cannot access PSUM on TRN2).
  6. fp16 stage -> HBM as fully-contiguous 4 KiB-per-partition slabs in
     device order [bb, (half,p), (g,e,d)]; the host reorders to
     [i, j, d] and upcasts to fp32.

Walrus's PE LDWEIGHTS struct carries at most ONE sync wait, so a
post-pass relocates excess waits onto InstNoOp instructions inserted
immediately before in the same engine stream.
"""

import sys

import numpy as np

for _p in ("/opt/trn_rl_repo", "/root/.axon_site/_ro/trn_rl_repo"):
    if _p not in sys.path:
        sys.path.append(_p)

B = 8
N = 512
K = 32
D = 64
CUTOFF = 10.0
DSHIFT = 6.0

CHUNK = 124          # d rows per chunk (partitions 124/125 = ones-rows)
NCHUNK = 5           # 124*4 + 16
HALVES_PER_CHUNK = CHUNK // 4  # 31

_CACHE = {}
TRACE = False  # set True (e.g. from test.py) to capture an NTFF profile


def _fix_waits(nc, mybir):
    """Enforce <=1 embedded sync wait on compute-engine instructions.

    Walrus's per-instruction ISA structs (PE S3_LW, DVE/ACT S2S2D2_*)
    carry a single sync-wait slot.  Excess waits move onto InstNoOp
    instructions inserted immediately before the instruction in the same
    engine stream — gating an earlier point of the same engine is
    strictly more conservative, and with no instruction in between it
    cannot deadlock.
    """
    limited = {
        mybir.EngineType.PE,
        mybir.EngineType.DVE,
        mybir.EngineType.Activation,
        mybir.EngineType.SP,
        mybir.EngineType.Pool,
    }
    for blk in nc.m.functions[0].blocks:
        insts = blk.instructions
        i = 0
        while i < len(insts):
            inst = insts[i]
            si = inst.sync_info
            if (
                inst.engine in limited
                and si is not None
                and si.on_wait
                and len(si.on_wait) > 1
            ):
                waits = list(si.on_wait)
                excess, keep = waits[:-1], waits[-1:]
                for w in excess:
                    nop = mybir.InstNoOp(
                        name=nc.get_next_instruction_name(),
                        sync_info=mybir.SyncInfo(on_wait=[w], on_update=[]),
                        bass_nofuse=True,
                        engine=inst.engine,
                    )
                    nc.register_instruction(nop)
                    insts.insert(i, nop)
                    i += 1
                si.on_wait = keep
            i += 1


def _half_chunk(hh):
    """half index (4 i-rows) -> (chunk c, local half index lr)."""
    if hh < 4 * HALVES_PER_CHUNK:
        return hh // HALVES_PER_CHUNK, hh % HALVES_PER_CHUNK
    return 4, hh - 4 * HALVES_PER_CHUNK


# ct16 column offsets: sel [128, 31*128], wc [128, 256], ones [124:126, 512]
C16_SEL = 0
C16_WC = HALVES_PER_CHUNK * 128          # 3968
C16_ONE = C16_WC + 256                   # 4224
CW16 = C16_ONE + 512                     # 4736


def _build_program():
    import concourse.bass as bass
    import concourse.tile as tile
    from concourse import mybir

    f32 = mybir.dt.float32
    f16 = mybir.dt.float16
    AF = mybir.ActivationFunctionType

    o = np.linspace(0.0, CUTOFF, K)
    coeff = float(-0.5 / (o[1] - o[0]) ** 2)
    s_scale = float(np.sqrt(-coeff))

    nc = bass.Bass("TRN2", target_bir_lowering=False, debug=False)

    ct32_d = nc.dram_tensor("ct32", [5, 1024], f32, kind="ExternalInput")
    ct16_d = nc.dram_tensor("ct16", [128, CW16], f16, kind="ExternalInput")
    out_d = nc.dram_tensor("out", [N // 8, 128, 2048], f16, kind="ExternalOutput")

    with tile.TileContext(nc) as tc:
        with (
            tc.tile_pool(name="consts", bufs=1) as consts,
            tc.tile_pool(name="dtile", bufs=1) as dpool,
            tc.tile_pool(name="work", bufs=4) as work,
            tc.tile_pool(name="stage", bufs=4) as stpool,
            tc.tile_pool(name="psA", bufs=2, space=bass.MemorySpace.PSUM) as psA,
            tc.tile_pool(name="psB", bufs=2, space=bass.MemorySpace.PSUM) as psB,
        ):
            ct32_s = consts.tile([128, 1024], f32, tag="ct32")
            ct16_s = consts.tile([128, CW16], f16, tag="ct16")
            a32 = ct32_d.ap()
            a16 = ct16_d.ap()
            nc.sync.dma_start(ct32_s[0:5, :], a32)
            # first sel blocks land before the bulk
            nc.sync.dma_start(ct16_s[:, 0:1024], a16[:, 0:1024])
            nc.sync.dma_start(ct16_s[:, 1024:CW16], a16[:, 1024:CW16])
            lg_s = ct32_s[0:5, 0:N]
            rg_s = ct32_s[0:5, N : 2 * N]
            sel_s = ct16_s[:, C16_SEL : C16_SEL + HALVES_PER_CHUNK * 128]
            wc_s = ct16_s[:, C16_WC : C16_WC + 256]

            # Phase 1: d chunks [124 rows, 512 j] fp16 (shifted by -6)
            dext = [
                dpool.tile([128, N], f16, name=f"dx{c}", tag=f"dx{c}")
                for c in range(NCHUNK)
            ]
            for c in range(NCHUNK):
                nr = CHUNK if c < 4 else N - 4 * CHUNK
                if nr < CHUNK:
                    # zero the unwritten rows so sel's 0.0 entries never
                    # multiply uninitialized NaN bits
                    nc.vector.memset(dext[c][:], 0.0)
                gps = psB.tile([128, 2 * N], f32, tag="eps")
                nc.tensor.matmul(
                    gps[0:nr, 0:N], lg_s[:, c * CHUNK : c * CHUNK + nr], rg_s
                )
                draw = work.tile([128, N], f32, tag="draw")
                nc.vector.tensor_scalar_max(draw[0:nr, :], gps[0:nr, 0:N], 0.0)
                dfull = work.tile([128, N], f32, tag="dfull")
                nc.scalar.activation(dfull[0:nr, :], draw[0:nr, :], AF.Sqrt)
                nc.vector.tensor_scalar_add(
                    dext[c][0:nr, :], dfull[0:nr, :], -DSHIFT
                )
                nc.sync.dma_start(
                    dext[c][CHUNK : CHUNK + 2, :],
                    a16[CHUNK : CHUNK + 2, C16_ONE : C16_ONE + 512],
                )

            # Phase 2
            diff_tiles = {}

            def emit_bcast(bb):
                diff = psA.tile([128, 2 * N], f32, tag="diff")
                for bi2 in range(2):
                    c, lr = _half_chunk(2 * bb + bi2)
                    nc.tensor.matmul(
                        diff[:, bi2 * N : (bi2 + 1) * N],
                        sel_s[0:126, lr * 128 : (lr + 1) * 128],
                        dext[c][0:126, :],
                    )
                diff_tiles[bb] = diff

            NBB = N // 8
            evac_ctr = 0
            emit_bcast(0)
            for bb in range(NBB):
                if bb + 1 < NBB:
                    emit_bcast(bb + 1)
                diff = diff_tiles.pop(bb)
                # rbf j-major, 1:1 with diff -> fully contiguous ACT write
                rbf = work.tile([128, 2 * N], f16, tag="rbf")
                nc.scalar.activation(
                    rbf[:], diff[:], AF.Derivative_Erf, scale=s_scale
                )

                # edge: W-half stationary, rbf streamed. out partitions =
                # (gw, d) for i_sub g = 2*wh + gw; free = j. Everything
                # contiguous; only 4 matmuls (2 ldweights targets) per bb.
                stage = stpool.tile([128, 2048], f16, tag="stage")
                for bi2 in range(2):
                    eps = psB.tile([128, 2 * N], f32, tag="eps")
                    for wh in range(2):
                        nc.tensor.matmul(
                            eps[:, wh * N : (wh + 1) * N],
                            wc_s[:, wh * 128 : (wh + 1) * 128],
                            rbf[:, bi2 * N : (bi2 + 1) * N],
                        )
                    # Pool cannot read PSUM; rotate the convert-copies over
                    # DVE (2/3) and ACT (1/3, alongside its Derivative_Erf)
                    dst = stage[:, bi2 * 1024 : (bi2 + 1) * 1024]
                    if evac_ctr % 3 == 2:
                        nc.scalar.activation(dst, eps[:], AF.Copy)
                    else:
                        nc.vector.tensor_copy(dst, eps[:])
                    evac_ctr += 1
                nc.sync.dma_start(out_d.ap()[bb], stage[:])

    _fix_waits(nc, mybir)
    return nc


def _host_inputs(pred_coords):
    x64 = pred_coords.astype(np.float64)  # [B, N, 3]
    r = (x64 * x64).sum(-1)  # [B, N]
    ones = np.ones((B, N), np.float64)
    lg = np.stack(
        [x64[:, :, 0], x64[:, :, 1], x64[:, :, 2], r, ones], axis=1
    ).astype(np.float32)  # [B, 5, N]
    rg = np.stack(
        [-2 * x64[:, :, 0], -2 * x64[:, :, 1], -2 * x64[:, :, 2], ones, r],
        axis=1,
    ).astype(np.float32)  # [B, 5, N]
    return lg, rg


def _host_consts(W):
    o = np.linspace(0.0, CUTOFF, K)

    sel = np.zeros((128, HALVES_PER_CHUNK * 128), np.float16)
    m = np.arange(128)
    bias = -(o[m % 32] - DSHIFT)  # f64
    b_hi = bias.astype(np.float16)
    b_lo = (bias - b_hi.astype(np.float64)).astype(np.float16)
    for lr in range(HALVES_PER_CHUNK):
        sel[4 * lr + m // 32, lr * 128 + m] = np.float16(1.0)
        sel[124, lr * 128 + m] = b_hi
        sel[125, lr * 128 + m] = b_lo

    # sqrt(pi)/2 compensates Derivative_Erf's 2/sqrt(pi) prefactor
    wc = np.zeros((128, 256), np.float16)
    wt = (W.astype(np.float64) * (np.sqrt(np.pi) / 2.0)).astype(np.float16)
    for g in range(4):
        wh, gw = divmod(g, 2)
        wc[32 * g : 32 * (g + 1), 128 * wh + 64 * gw : 128 * wh + 64 * gw + 64] = wt.T

    ct16 = np.zeros((128, CW16), np.float16)
    ct16[:, C16_SEL : C16_SEL + HALVES_PER_CHUNK * 128] = sel
    ct16[:, C16_WC : C16_WC + 256] = wc
    ct16[124:126, C16_ONE : C16_ONE + 512] = np.float16(1.0)
    return ct16


def kernel(pred_coords, mask, W, b):
    from concourse.bass_utils import run_bass_kernel_spmd

    pred_coords = np.asarray(pred_coords)
    mask = np.asarray(mask)
    W = np.asarray(W)
    b = np.asarray(b)

    if "nc" not in _CACHE:
        _CACHE["nc"] = _build_program()
    nc = _CACHE["nc"]

    lg, rg = _host_inputs(pred_coords)
    ct16 = _host_consts(W)
    in_maps = []
    for cidx in range(B):
        ct32 = np.concatenate([lg[cidx], rg[cidx]], axis=1).astype(np.float32)
        in_maps.append({"ct32": ct32, "ct16": ct16})
    import os

    tdir = os.environ.get("KTRACE_DIR") or None
    res = run_bass_kernel_spmd(
        nc, in_maps, list(range(B)), trace=TRACE, tmpdir=tdir
    )
    _CACHE["last_res"] = res
    # device order [bb, (bi2, p), (g, e, d)] -> [i, j, d]
    outs = []
    for c in range(B):
        arr = np.asarray(res.results[c]["out"])  # [64, 128, 2048] f16
        arr = arr.reshape(64, 2, 64, 2, 2, 512)  # bb, gw, d, bi2, wh, j
        arr = arr.transpose(0, 3, 4, 1, 5, 2).reshape(N, N, D)
        outs.append(arr)
    out = np.stack(outs).astype(np.float32) + b.astype(np.float32)

    if not np.all(mask == 1.0):
        adj = (mask[:, None, :] * mask[:, :, None]).astype(np.float32)
        out = out * adj[..., None]
    return out


# revision 10
# speedup vs baseline: 2.4909x; 1.0060x over previous
"""Trainium2 Bass kernel for nn_DistanceEdgeSelfCond.

Computes, for inputs pred_coords [8,512,3], mask [8,512], W [64,32], b [64]:
    d[i,j]   = ||x_i - x_j||                        (pairwise distances)
    rbf      = exp(coeff * (d - o_k)^2)             (gaussian smearing, K=32)
    edge     = rbf @ W.T + b                        ([B,512,512,64])
    out      = edge * (mask_i * mask_j)[...,None]

Sharding: data-parallel over B — one batch per NeuronCore (8 cores).

Device pipeline (per core):
  1. Gram matmul (fp32) with host-augmented [5, nc] factors -> d^2 chunks
     of 124 i-rows; DVE relu + ACT sqrt -> d fp32; Pool shifts by -6 and
     casts to fp16 (the shift centers the rbf-active region so fp16
     rounding of d stays ~1.6e-3 there). Partitions 124/125 of each
     chunk hold constant ones-rows (filled by DMA; engine ops cannot
     start at partition 124).
  2. Per 4-i-row half: ONE fp16 broadcast matmul. The select matrix
     carries 1.0 indicators on the d-rows plus hi/lo halves of
     -(o_k - 6) against the two ones-rows, so PSUM receives
     diff = d - o_k (exact center) replicated over the 32 rbf
     channels: [(i_sub,k), j].
  3. ACT Derivative_Erf: d/dx erf = 2/sqrt(pi) * exp(-x^2), so ONE
     activation with scale=sqrt(-coeff) turns diff directly into
     (2/sqrt(pi)) * rbf in fp16 — no separate square or exp pass.
     The sqrt(pi)/2 factor is folded into W on the host.
  4. Edge matmul fp16 per pixel-offset e in 0..7: lhsT = rbf block e
     (columns stored e-major so weights APs stay contiguous), rhs =
     block-diagonal W' -> out [(half,p), (i_sub,d)] fp32 PSUM; each
     output partition owns 8 consecutive pixels.
  5. Evac = pure fp32->fp16 convert copies split DVE/ACT (bias b is
     added on the host; Pool/GPSIMD cannot access PSUM on TRN2).
  6. fp16 stage -> HBM as fully-contiguous 4 KiB-per-partition slabs in
     device order [bb, (half,p), (g,e,d)]; the host reorders to
     [i, j, d] and upcasts to fp32.

Walrus's PE LDWEIGHTS struct carries at most ONE sync wait, so a
post-pass relocates excess waits onto InstNoOp instructions inserted
immediately before in the same engine stream.
"""

import sys

import numpy as np

for _p in ("/opt/trn_rl_repo", "/root/.axon_site/_ro/trn_rl_repo"):
    if _p not in sys.path:
        sys.path.append(_p)

B = 8
N = 512
K = 32
D = 64
CUTOFF = 10.0
DSHIFT = 6.0

CHUNK = 124          # d rows per chunk (partitions 124/125 = ones-rows)
NCHUNK = 5           # 124*4 + 16
HALVES_PER_CHUNK = CHUNK // 4  # 31

_CACHE = {}
TRACE = False  # set True (e.g. from test.py) to capture an NTFF profile


def _fix_waits(nc, mybir):
    """Enforce <=1 embedded sync wait on compute-engine instructions.

    Walrus's per-instruction ISA structs (PE S3_LW, DVE/ACT S2S2D2_*)
    carry a single sync-wait slot.  Excess waits move onto InstNoOp
    instructions inserted immediately before the instruction in the same
    engine stream — gating an earlier point of the same engine is
    strictly more conservative, and with no instruction in between it
    cannot deadlock.
    """
    limited = {
        mybir.EngineType.PE,
        mybir.EngineType.DVE,
        mybir.EngineType.Activation,
        mybir.EngineType.SP,
        mybir.EngineType.Pool,
    }
    for blk in nc.m.functions[0].blocks:
        insts = blk.instructions
        i = 0
        while i < len(insts):
            inst = insts[i]
            si = inst.sync_info
            if (
                inst.engine in limited
                and si is not None
                and si.on_wait
                and len(si.on_wait) > 1
            ):
                waits = list(si.on_wait)
                excess, keep = waits[:-1], waits[-1:]
                for w in excess:
                    nop = mybir.InstNoOp(
                        name=nc.get_next_instruction_name(),
                        sync_info=mybir.SyncInfo(on_wait=[w], on_update=[]),
                        bass_nofuse=True,
                        engine=inst.engine,
                    )
                    nc.register_instruction(nop)
                    insts.insert(i, nop)
                    i += 1
                si.on_wait = keep
            i += 1


def _half_chunk(hh):
    """half index (4 i-rows) -> (chunk c, local half index lr)."""
    if hh < 4 * HALVES_PER_CHUNK:
        return hh // HALVES_PER_CHUNK, hh % HALVES_PER_CHUNK
    return 4, hh - 4 * HALVES_PER_CHUNK


# ct16 column offsets: sel [128, 31*128], wc [128, 256], ones [124:126, 512]
C16_SEL = 0
C16_WC = HALVES_PER_CHUNK * 128          # 3968
C16_ONE = C16_WC + 256                   # 4224
CW16 = C16_ONE + 512                     # 4736


def _build_program():
    import concourse.bass as bass
    import concourse.tile as tile
    from concourse import mybir

    f32 = mybir.dt.float32
    f16 = mybir.dt.float16
    AF = mybir.ActivationFunctionType

    o = np.linspace(0.0, CUTOFF, K)
    coeff = float(-0.5 / (o[1] - o[0]) ** 2)
    s_scale = float(np.sqrt(-coeff))

    nc = bass.Bass("TRN2", target_bir_lowering=False, debug=False)

    ct32_d = nc.dram_tensor("ct32", [5, 1024], f32, kind="ExternalInput")
    ct16_d = nc.dram_tensor("ct16", [128, CW16], f16, kind="ExternalInput")
    out_d = nc.dram_tensor("out", [N // 8, 128, 2048], f16, kind="ExternalOutput")

    with tile.TileContext(nc) as tc:
        with (
            tc.tile_pool(name="consts", bufs=1) as consts,
            tc.tile_pool(name="dtile", bufs=1) as dpool,
            tc.tile_pool(name="work", bufs=4) as work,
            tc.tile_pool(name="stage", bufs=4) as stpool,
            tc.tile_pool(name="psA", bufs=2, space=bass.MemorySpace.PSUM) as psA,
            tc.tile_pool(name="psB", bufs=2, space=bass.MemorySpace.PSUM) as psB,
        ):
            ct32_s = consts.tile([128, 1024], f32, tag="ct32")
            ct16_s = consts.tile([128, CW16], f16, tag="ct16")
            a32 = ct32_d.ap()
            a16 = ct16_d.ap()
            nc.sync.dma_start(ct32_s[0:5, :], a32)
            # first sel blocks land before the bulk
            nc.sync.dma_start(ct16_s[:, 0:1024], a16[:, 0:1024])
            nc.sync.dma_start(ct16_s[:, 1024:CW16], a16[:, 1024:CW16])
            lg_s = ct32_s[0:5, 0:N]
            rg_s = ct32_s[0:5, N : 2 * N]
            sel_s = ct16_s[:, C16_SEL : C16_SEL + HALVES_PER_CHUNK * 128]
            wc_s = ct16_s[:, C16_WC : C16_WC + 256]

            # Phase 1: d chunks [124 rows, 512 j] fp16 (shifted by -6)
            dext = [
                dpool.tile([128, N], f16, name=f"dx{c}", tag=f"dx{c}")
                for c in range(NCHUNK)
            ]
            for c in range(NCHUNK):
                nr = CHUNK if c < 4 else N - 4 * CHUNK
                if nr < CHUNK:
                    # zero the unwritten rows so sel's 0.0 entries never
                    # multiply uninitialized NaN bits
                    nc.vector.memset(dext[c][:], 0.0)
                gps = psB.tile([128, 2 * N], f32, tag="eps")
                nc.tensor.matmul(
                    gps[0:nr, 0:N], lg_s[:, c * CHUNK : c * CHUNK + nr], rg_s
                )
                draw = work.tile([128, N], f32, tag="draw")
                nc.vector.tensor_scalar_max(draw[0:nr, :], gps[0:nr, 0:N], 0.0)
                dfull = work.tile([128, N], f32, tag="dfull")
                nc.scalar.activation(dfull[0:nr, :], draw[0:nr, :], AF.Sqrt)
                nc.vector.tensor_scalar_add(
                    dext[c][0:nr, :], dfull[0:nr, :], -DSHIFT
                )
                nc.sync.dma_start(
                    dext[c][CHUNK : CHUNK + 2, :],
                    a16[CHUNK : CHUNK + 2, C16_ONE : C16_ONE + 512],
                )

            # Phase 2
            diff_tiles = {}

            def emit_bcast(bb):
                diff = psA.tile([128, 2 * N], f32, tag="diff")
                for bi2 in range(2):
                    c, lr = _half_chunk(2 * bb + bi2)
                    nc.tensor.matmul(
                        diff[:, bi2 * N : (bi2 + 1) * N],
                        sel_s[0:126, lr * 128 : (lr + 1) * 128],
                        dext[c][0:126, :],
                    )
                diff_tiles[bb] = diff

            NBB = N // 8
            evac_ctr = 0
            rbf_tiles = {}

            def emit_rbf(bb):
                # derf runs one bb ahead of the edge matmuls so ACT's
                # evac work never blocks the PE->ACT->PE chain
                diff = diff_tiles.pop(bb)
                rbf = work.tile([128, 2 * N], f16, tag="rbf")
                nc.scalar.activation(
                    rbf[:], diff[:], AF.Derivative_Erf, scale=s_scale
                )
                rbf_tiles[bb] = rbf

            emit_bcast(0)
            emit_rbf(0)
            for bb in range(NBB):
                if bb + 1 < NBB:
                    emit_bcast(bb + 1)
                    emit_rbf(bb + 1)
                rbf = rbf_tiles.pop(bb)

                # edge: W-half stationary, rbf streamed. out partitions =
                # (gw, d) for i_sub g = 2*wh + gw; free = j. Everything
                # contiguous; wh-outer order keeps identical ldweights
                # back-to-back.
                stage = stpool.tile([128, 2048], f16, tag="stage")
                eps = [
                    psB.tile([128, 2 * N], f32, name=f"eps{i}", tag="eps")
                    for i in range(2)
                ]
                for wh in range(2):
                    for bi2 in range(2):
                        nc.tensor.matmul(
                            eps[bi2][:, wh * N : (wh + 1) * N],
                            wc_s[:, wh * 128 : (wh + 1) * 128],
                            rbf[:, bi2 * N : (bi2 + 1) * N],
                        )
                for bi2 in range(2):
                    # Pool cannot read PSUM; rotate the convert-copies over
                    # DVE (2/3) and ACT (1/3, alongside its Derivative_Erf)
                    dst = stage[:, bi2 * 1024 : (bi2 + 1) * 1024]
                    if evac_ctr % 3 == 2:
                        nc.scalar.activation(dst, eps[bi2][:], AF.Copy)
                    else:
                        nc.vector.tensor_copy(dst, eps[bi2][:])
                    evac_ctr += 1
                nc.sync.dma_start(out_d.ap()[bb], stage[:])

    _fix_waits(nc, mybir)
    return nc


def _host_inputs(pred_coords):
    x64 = pred_coords.astype(np.float64)  # [B, N, 3]
    r = (x64 * x64).sum(-1)  # [B, N]
    ones = np.ones((B, N), np.float64)
    lg = np.stack(
        [x64[:, :, 0], x64[:, :, 1], x64[:, :, 2], r, ones], axis=1
    ).astype(np.float32)  # [B, 5, N]
    rg = np.stack(
        [-2 * x64[:, :, 0], -2 * x64[:, :, 1], -2 * x64[:, :, 2], ones, r],
        axis=1,
    ).astype(np.float32)  # [B, 5, N]
    return lg, rg


def _host_consts(W):
    o = np.linspace(0.0, CUTOFF, K)

    sel = np.zeros((128, HALVES_PER_CHUNK * 128), np.float16)
    m = np.arange(128)
    bias = -(o[m % 32] - DSHIFT)  # f64
    b_hi = bias.astype(np.float16)
    b_lo = (bias - b_hi.astype(np.float64)).astype(np.float16)
    for lr in range(HALVES_PER_CHUNK):
        sel[4 * lr + m // 32, lr * 128 + m] = np.float16(1.0)
        sel[124, lr * 128 + m] = b_hi
        sel[125, lr * 128 + m] = b_lo

    # sqrt(pi)/2 compensates Derivative_Erf's 2/sqrt(pi) prefactor
    wc = np.zeros((128, 256), np.float16)
    wt = (W.astype(np.float64) * (np.sqrt(np.pi) / 2.0)).astype(np.float16)
    for g in range(4):
        wh, gw = divmod(g, 2)
        wc[32 * g : 32 * (g + 1), 128 * wh + 64 * gw : 128 * wh + 64 * gw + 64] = wt.T

    ct16 = np.zeros((128, CW16), np.float16)
    ct16[:, C16_SEL : C16_SEL + HALVES_PER_CHUNK * 128] = sel
    ct16[:, C16_WC : C16_WC + 256] = wc
    ct16[124:126, C16_ONE : C16_ONE + 512] = np.float16(1.0)
    return ct16


def kernel(pred_coords, mask, W, b):
    from concourse.bass_utils import run_bass_kernel_spmd

    pred_coords = np.asarray(pred_coords)
    mask = np.asarray(mask)
    W = np.asarray(W)
    b = np.asarray(b)

    if "nc" not in _CACHE:
        _CACHE["nc"] = _build_program()
    nc = _CACHE["nc"]

    lg, rg = _host_inputs(pred_coords)
    ct16 = _host_consts(W)
    in_maps = []
    for cidx in range(B):
        ct32 = np.concatenate([lg[cidx], rg[cidx]], axis=1).astype(np.float32)
        in_maps.append({"ct32": ct32, "ct16": ct16})
    import os

    tdir = os.environ.get("KTRACE_DIR") or None
    res = run_bass_kernel_spmd(
        nc, in_maps, list(range(B)), trace=TRACE, tmpdir=tdir
    )
    _CACHE["last_res"] = res
    # device order [bb, (bi2, p), (g, e, d)] -> [i, j, d]
    outs = []
    for c in range(B):
        arr = np.asarray(res.results[c]["out"])  # [64, 128, 2048] f16
        arr = arr.reshape(64, 2, 64, 2, 2, 512)  # bb, gw, d, bi2, wh, j
        arr = arr.transpose(0, 3, 4, 1, 5, 2).reshape(N, N, D)
        outs.append(arr)
    out = np.stack(outs).astype(np.float32) + b.astype(np.float32)

    if not np.all(mask == 1.0):
        adj = (mask[:, None, :] * mask[:, :, None]).astype(np.float32)
        out = out * adj[..., None]
    return out
